# revision 19
# baseline (speedup 1.0000x reference)
"""HGT layer kernel for 8 Trainium2 NeuronCores.

Strategy (dst-sharded graph parallel, transfer-minimized):
  - Host folds relation transforms / priors / skip gate into effective
    weights. h ships as fp8(e3m4, x2-scaled, compensated in the weights),
    transposed; it is widened to fp16 on device for the projections. The
    skip residual (1-alpha)h + alpha*ba is added on the host, where h is
    already resident in fp32.
  - The merged projection weights ship sharded: each core uploads 1/8th and
    the full matrix is rebuilt with a small AllGather before projections.
  - Each core owns N/8=2500 destination nodes and their incoming edges;
    nodes are first-fit-decreasing packed into NW windows of <=128 dst
    nodes / 2048 edge slots.
  - Device: project q/kv for own nodes (fp16), AllGather kv table, then per
    window: dma_gather kv[src] and q[dst] rows, DVE dot-product scores, ACT
    exp, PE onehot-matmul aggregation of [messages | exp] into PSUM,
    normalize, flush. The onehot is built on device (iota + is_equal
    against per-slot column ids), not uploaded.
  - Final: transpose-gather normalized agg; the output projection is
    computed transposed (dout on partitions) and ships back as fp8
    (x16-scaled) [2,128,2500]; host adds the residual and unshards.
  - ALL per-core inputs ship as ONE f16 blob parameter (fp8/int16 sections
    are bitcast-sliced on device); per-array tunnel latency dominates, so
    fewer/larger transfers win. Total traffic ~18MB/call vs 182MB naive.
"""

import math
import os
import numpy as np
import ml_dtypes

import jax

jax.config.update(
    "jax_compilation_cache_dir",
    os.path.join(os.environ.get("TMPDIR", "/tmp"), "bass_hgt_jax_cache"),
)
jax.config.update("jax_persistent_cache_min_compile_time_secs", 0.0)

import concourse.bacc as bacc
import concourse.tile as tile
from concourse import mybir
from concourse.bass_utils import run_bass_kernel_spmd

N = 20000
E = 320000
D = 256
H = 8
DK = 32
NCORES = 8
NPC = N // NCORES          # 2500 nodes per core
NTN = 2560                 # padded nodes per core (20 tiles of 128)
NTILES = NTN // 128        # 20
WSLOTS = 2048              # edge slots per window
WCH = WSLOTS // 128        # 16 chunks per window
WSPAN = 128                # max dst nodes per window
FCH = 512                  # node columns per final-phase chunk
H8SCALE = 2.0              # fp8 pre-scale for h (compensated in wq/wkv)
H8MAX = 15.5               # float8_e3m4 max finite; clip before cast
OSCALE = 16.0              # fp8 scale for the trans output (undone on host)

F16 = mybir.dt.float16
F32 = mybir.dt.float32
F8 = mybir.dt.float8e3
I16 = mybir.dt.int16

# blob section offsets in f16 units
O_HT = 0                                   # [2][128, NPC] fp8
SZ_HT_J = 128 * NPC // 2                   # one j-plane, f16 units
O_WZS = O_HT + 2 * SZ_HT_J                 # [2][128, 128] f16
SZ_WZS_J = 128 * 128

_cache = {}
_graph_cache = {}
LAST_RESULTS = None
LAST_EXEC_NS = None

# fp8(e3m4) byte -> f32/OSCALE lookup for fast output decode
_OLUT = None
# f16 bits -> clipped e3m4 byte lookup for fast h encode
_HLUT = None


def _olut():
    global _OLUT
    if _OLUT is None:
        _OLUT = (
            np.arange(256, dtype=np.uint8).view(ml_dtypes.float8_e3m4)
            .astype(np.float32) * (1.0 / OSCALE)
        )
    return _OLUT


def _hlut():
    global _HLUT
    if _HLUT is None:
        v = np.arange(65536, dtype=np.uint16).view(np.float16).astype(np.float32)
        v = np.clip(np.nan_to_num(v, nan=0.0, posinf=H8MAX, neginf=-H8MAX),
                    -H8MAX, H8MAX)
        _HLUT = v.astype(ml_dtypes.float8_e3m4).view(np.uint8)
    return _HLUT


def _build(NW, use_bias):
    IDXL = 2 * NW * 128 + NTN // 16
    QOFF = NW * 128
    VOFF = 2 * NW * 128
    O_IDX = O_WZS + 2 * SZ_WZS_J           # [16, IDXL] i16
    SZ_IDX = 16 * IDXL
    O_COLV = O_IDX + SZ_IDX                # [128, NW*WCH] f16
    SZ_COLV = 128 * NW * WCH
    O_BZ = O_COLV + SZ_COLV                # [1, 3D] f16 (optional)
    BLOB = O_BZ + (3 * D if use_bias else 0)

    nc = bacc.Bacc()
    blob = nc.declare_dram_parameter("blob", [BLOB], F16, isOutput=False)
    outp = nc.declare_dram_parameter("out", [2, 128, NPC], F8, isOutput=True)

    with tile.TileContext(nc) as tc:
        with (
            tc.tile_pool(name="const", bufs=1) as constp,
            tc.tile_pool(name="dram", bufs=1, space="DRAM") as dram,
            tc.tile_pool(name="proj", bufs=3) as projp,
            tc.tile_pool(name="psum", bufs=2, space="PSUM") as psump,
            tc.tile_pool(name="edge", bufs=2) as edgep,
            tc.tile_pool(name="fin", bufs=2) as finp,
        ):
            q_tab = dram.tile([NTN, D], F16)
            kv_slice = dram.tile([NTN, 2 * D], F16)
            kv_full = nc.dram_tensor(
                "kv_full", [NCORES * NTN, 2 * D], F16, addr_space="Shared")
            vn = dram.tile([NW * 128, D], F16)
            wzs_dram = dram.tile([2, 128, 128], F16)
            wz_full = nc.dram_tensor(
                "wz_full", [2 * NCORES, 128, 128], F16, addr_space="Shared")

            # ---- weight shard AllGather ----
            wzs_sb = constp.tile([128, 2, 128], F16)
            for j in (0, 1):
                nc.sync.dma_start(
                    wzs_sb[:, j, :],
                    blob[O_WZS + j * SZ_WZS_J:O_WZS + (j + 1) * SZ_WZS_J]
                    .rearrange("(p l) -> p l", p=128))
                nc.sync.dma_start(wzs_dram[j], wzs_sb[:, j, :])
            nc.gpsimd.collective_compute(
                "AllGather",
                mybir.AluOpType.bypass,
                replica_groups=[list(range(NCORES))],
                ins=[wzs_dram.opt()],
                outs=[wz_full[:]],
            )
            wz_sb = constp.tile([128, 2, 4 * D], F16)
            for c8 in range(NCORES):
                for j in (0, 1):
                    nc.sync.dma_start(
                        wz_sb[:, j, c8 * 128:(c8 + 1) * 128], wz_full[c8 * 2 + j])

            # ---- constants ----
            hT8_sb = constp.tile([128, 2, NPC], F8)
            for j in (0, 1):
                nc.sync.dma_start(
                    hT8_sb[:, j, :],
                    blob[O_HT + j * SZ_HT_J:O_HT + (j + 1) * SZ_HT_J]
                    .bitcast(F8).rearrange("(p l) -> p l", p=128))
            hT_sb = constp.tile([128, 2, NTN], F16)
            nc.vector.memset(hT_sb[:], 0.0)
            for j in (0, 1):
                nc.vector.tensor_copy(hT_sb[:, j, 0:NPC], hT8_sb[:, j, :])
            idx_sb = constp.tile([128, IDXL], I16)
            idx_src = blob[O_IDX:O_IDX + SZ_IDX].bitcast(I16).rearrange(
                "(p l) -> p l", p=16)
            for a in range(8):
                nc.sync.dma_start(idx_sb[16 * a:16 * (a + 1), :], idx_src)
            colv_sb = constp.tile([128, NW * WCH], F16)
            nc.sync.dma_start(
                colv_sb[:],
                blob[O_COLV:O_COLV + SZ_COLV].rearrange("(p l) -> p l", p=128))
            iota_big = constp.tile([128, WCH, 128], F16)
            nc.gpsimd.iota(
                iota_big[:], pattern=[[0, WCH], [1, 128]], base=0,
                channel_multiplier=0, allow_small_or_imprecise_dtypes=True)
            if use_bias:
                ones_sb = constp.tile([1, 128], F16)
                nc.vector.memset(ones_sb[:], 1.0)
                bz_sb = constp.tile([1, 3 * D], F16)
                nc.sync.dma_start(
                    bz_sb[:], blob[O_BZ:O_BZ + 3 * D].rearrange("(p l) -> p l", p=1))

            # ---- projection phase ----
            for nt in range(NTILES):
                sl = slice(nt * 128, (nt + 1) * 128)
                pkv = psump.tile([128, 2 * D], F32, tag="pkv")
                for j in (0, 1):
                    nc.tensor.matmul(
                        pkv[:], hT_sb[:, j, sl], wz_sb[:, j, D:3 * D],
                        start=(j == 0), stop=(j == 1 and not use_bias),
                    )
                if use_bias:
                    nc.tensor.matmul(
                        pkv[:], ones_sb[:], bz_sb[:, D:3 * D], start=False, stop=True)
                kv_sb = projp.tile([128, 2 * D], F16, tag="kv")
                nc.vector.tensor_copy(kv_sb[:], pkv[:])
                nc.sync.dma_start(kv_slice[sl, :], kv_sb[:])

                pq = psump.tile([128, D], F32, tag="pq")
                for j in (0, 1):
                    nc.tensor.matmul(
                        pq[:], hT_sb[:, j, sl], wz_sb[:, j, 0:D],
                        start=(j == 0), stop=(j == 1 and not use_bias),
                    )
                if use_bias:
                    nc.tensor.matmul(
                        pq[:], ones_sb[:], bz_sb[:, 0:D], start=False, stop=True)
                q_sb = projp.tile([128, D], F16, tag="q")
                nc.vector.tensor_copy(q_sb[:], pq[:])
                nc.sync.dma_start(q_tab[sl, :], q_sb[:])

            nc.gpsimd.collective_compute(
                "AllGather",
                mybir.AluOpType.bypass,
                replica_groups=[list(range(NCORES))],
                ins=[kv_slice.opt()],
                outs=[kv_full[:]],
            )

            # ---- edge phase ----
            for w in range(NW):
                csl = slice(w * 128, (w + 1) * 128)
                kvg = edgep.tile([128, WCH, 2 * D], F16, tag="kvg")
                nc.gpsimd.dma_gather(
                    kvg[:], kv_full[:], idx_sb[:, csl],
                    num_idxs=WSLOTS, num_idxs_reg=WSLOTS, elem_size=2 * D,
                    single_packet=False,
                )
                qg = edgep.tile([128, WCH, D], F16, tag="qg")
                nc.gpsimd.dma_gather(
                    qg[:], q_tab[:], idx_sb[:, QOFF + w * 128:QOFF + (w + 1) * 128],
                    num_idxs=WSLOTS, num_idxs_reg=WSLOTS, elem_size=D,
                    single_packet=False,
                )
                oa_sb = edgep.tile([128, WCH, 128], F16, tag="oa")
                nc.vector.tensor_tensor(
                    oa_sb[:],
                    colv_sb[:, w * WCH:(w + 1) * WCH].broadcast_to([128, WCH, 128]),
                    iota_big[:],
                    op=mybir.AluOpType.is_equal,
                )

                prod = edgep.tile([128, WCH, D], F16, tag="prod")
                nc.vector.tensor_mul(prod[:], qg[:], kvg[:, :, 0:D])
                scores = edgep.tile([128, WCH, H], F32, tag="sc")
                nc.vector.tensor_reduce(
                    scores[:],
                    prod[:].rearrange("p c (h k) -> p c h k", h=H),
                    axis=mybir.AxisListType.X,
                    op=mybir.AluOpType.add,
                )
                msgz = edgep.tile([128, WCH, D + H], F16, tag="msgz")
                nc.scalar.activation(
                    msgz[:, :, D:D + H], scores[:], mybir.ActivationFunctionType.Exp
                )
                nc.vector.tensor_mul(
                    msgz[:, :, 0:D].rearrange("p c (h k) -> p c h k", h=H),
                    kvg[:, :, D:2 * D].rearrange("p c (h k) -> p c h k", h=H),
                    msgz[:, :, D:D + H].broadcast_to([128, WCH, H, DK]),
                )
                pw = psump.tile([128, D + H], F32, tag="pw")
                for i in range(WCH):
                    nc.tensor.matmul(
                        pw[:], oa_sb[:, i, :], msgz[:, i, :],
                        start=(i == 0), stop=(i == WCH - 1),
                    )
                zr = finp.tile([128, H], F32, tag="zr")
                nc.vector.tensor_scalar_add(zr[:], pw[:, D:D + H], 1e-30)
                zrec = finp.tile([128, H], F32, tag="zrec")
                nc.vector.reciprocal(zrec[:], zr[:])
                vb = finp.tile([128, D], F16, tag="vb")
                nc.vector.tensor_mul(
                    vb[:].rearrange("p (h k) -> p h k", h=H),
                    pw[:, 0:D].rearrange("p (h k) -> p h k", h=H),
                    zrec[:].broadcast_to([128, H, DK]),
                )
                nc.sync.dma_start(vn[csl, :], vb[:])

            # ---- final phase (transposed: dout on partitions) ----
            tg = constp.tile([128, 2, NTN], F16)
            nc.gpsimd.dma_gather(
                tg[:], vn[:], idx_sb[:, VOFF:VOFF + NTN // 16],
                num_idxs=NTN, num_idxs_reg=NTN, elem_size=D, transpose=True,
                single_packet=False,
            )
            for half in (0, 1):
                wsl = slice(3 * D + half * 128, 3 * D + (half + 1) * 128)
                for c0 in range(0, NPC, FCH):
                    c1 = min(c0 + FCH, NPC)
                    cw = c1 - c0
                    po = psump.tile([128, FCH], F32, tag="po")
                    for j in (0, 1):
                        nc.tensor.matmul(
                            po[:, 0:cw], wz_sb[:, j, wsl], tg[:, j, c0:c1],
                            start=(j == 0), stop=(j == 1),
                        )
                    # scale into fp8 range, clamp both sides, cast to e3m4
                    oth = finp.tile([128, FCH], F16, tag="oth")
                    nc.vector.tensor_scalar(
                        oth[:, 0:cw], po[:, 0:cw], OSCALE, H8MAX,
                        op0=mybir.AluOpType.mult, op1=mybir.AluOpType.min,
                    )
                    ot8 = finp.tile([128, FCH], F8, tag="ot8")
                    nc.vector.tensor_scalar_max(ot8[:, 0:cw], oth[:, 0:cw], -H8MAX)
                    nc.sync.dma_start(outp[half, :, c0:c1], ot8[:, 0:cw])

    nc.compile()
    return nc


def _wrap16(v):
    """[L] int array -> [16, L//16] wrapped int16 (16-partition wrap)."""
    L = v.shape[0]
    return np.ascontiguousarray(v.reshape(L // 16, 16).T.astype(np.int16))


def _wrap16_win(v):
    """[NW, WSLOTS] -> [16, NW*128]: per-window wrapped layout."""
    NW = v.shape[0]
    return np.ascontiguousarray(
        v.reshape(NW, WSLOTS // 16, 16)
        .transpose(2, 0, 1)
        .reshape(16, NW * (WSLOTS // 16))
        .astype(np.int16)
    )


def _pack_windows(degs):
    """Next-fit-decreasing pack nodes into windows of <=WSPAN nodes /
    <=WSLOTS slots. Returns (win_of, col_of, n_windows)."""
    npc = degs.shape[0]
    assert degs.max() <= WSLOTS, "node degree exceeds window slot capacity"
    order = np.argsort(-degs, kind="stable")
    cum = np.cumsum(degs[order])
    win_of_s = np.empty(npc, np.int64)
    col_of_s = np.empty(npc, np.int64)
    start = 0
    base = 0
    w = 0
    while start < npc:
        hi = min(start + WSPAN, npc)
        m = int(np.searchsorted(cum[start:hi], base + WSLOTS, side="right"))
        assert m > 0
        win_of_s[start:start + m] = w
        col_of_s[start:start + m] = np.arange(m)
        base = int(cum[start + m - 1])
        start += m
        w += 1
    win_of = np.empty(npc, np.int64)
    col_of = np.empty(npc, np.int64)
    win_of[order] = win_of_s
    col_of[order] = col_of_s
    return win_of, col_of, w


def kernel(h, src, dst, Wk, bk, Wq, bq, Wv, bv, Wa, ba, rel_att, rel_msg, rel_pri, skip):
    global LAST_RESULTS, LAST_EXEC_NS
    h = np.asarray(h, np.float32)
    src = np.asarray(src, np.int32)
    dst = np.asarray(dst, np.int32)

    # ---- fold weights on host ----
    scale = (np.asarray(rel_pri, np.float32) / math.sqrt(DK)).astype(np.float32)
    WqT = np.asarray(Wq, np.float32).T.reshape(D, H, DK)
    Wq_eff = (WqT * scale[None, :, None]).reshape(D, D)
    bq_eff = (np.asarray(bq, np.float32).reshape(H, DK) * scale[:, None]).reshape(D)
    WkT = np.asarray(Wk, np.float32).T.reshape(D, H, DK)
    Wk_eff = np.einsum("dhk,hke->dhe", WkT, np.asarray(rel_att, np.float32)).reshape(D, D)
    bk_eff = np.einsum("hk,hke->he", np.asarray(bk, np.float32).reshape(H, DK),
                       np.asarray(rel_att, np.float32)).reshape(D)
    WvT = np.asarray(Wv, np.float32).T.reshape(D, H, DK)
    Wv_eff = np.einsum("dhk,hke->dhe", WvT, np.asarray(rel_msg, np.float32)).reshape(D, D)
    bv_eff = np.einsum("hk,hke->he", np.asarray(bv, np.float32).reshape(H, DK),
                       np.asarray(rel_msg, np.float32)).reshape(D)
    Wkv_eff = np.concatenate([Wk_eff, Wv_eff], axis=1)          # [256, 512]
    bkv_eff = np.concatenate([bk_eff, bv_eff])                  # [512]
    alpha = float(1.0 / (1.0 + math.exp(-float(np.asarray(skip)))))
    ra = 1.0 - alpha
    Wa_eff = (alpha * np.asarray(Wa, np.float32).T)             # [256, 256]
    use_bias = bool(np.any(bq_eff) or np.any(bkv_eff))

    # ---- edge preprocessing (memoized on graph content) ----
    import hashlib
    gk = hashlib.blake2b(src.tobytes(), digest_size=16)
    gk.update(dst.tobytes())
    gkey = gk.digest()
    if gkey not in _graph_cache:
        order = np.argsort(dst, kind="stable")
        dsts = dst[order]
        srcs = src[order]
        core_of = dsts // NPC
        core_starts = np.searchsorted(core_of, np.arange(NCORES + 1))
        deg = np.bincount(dst, minlength=N)

        packs = []
        NW = 0
        for c in range(NCORES):
            win_of, col_of, nw = _pack_windows(deg[c * NPC:(c + 1) * NPC])
            packs.append((win_of, col_of))
            NW = max(NW, nw)

        f16 = np.float16
        idxcolv = []
        for c in range(NCORES):
            n0 = c * NPC
            e0, e1 = core_starts[c], core_starts[c + 1]
            ed = dsts[e0:e1] - n0         # local dst
            es = srcs[e0:e1]              # global src
            win_of, col_of = packs[c]
            wid = win_of[ed]
            # slot assignment: within window, sort by src kv row for locality
            es_row = (es // NPC) * NTN + (es % NPC)
            sort2 = np.lexsort((es_row, wid))
            ed = ed[sort2]
            es_row = es_row[sort2]
            wid = wid[sort2]
            # rank within window
            wcounts = np.bincount(wid, minlength=NW)
            woff = np.zeros(NW + 1, np.int64)
            np.cumsum(wcounts, out=woff[1:])
            rank = np.arange(e1 - e0) - woff[wid]
            slot = wid * WSLOTS + rank    # global slot in [0, NW*WSLOTS)

            src_slots = np.zeros((NW, WSLOTS), np.int64)
            q_slots = np.zeros((NW, WSLOTS), np.int64)
            src_slots.reshape(-1)[slot] = es_row
            q_slots.reshape(-1)[slot] = ed
            # per-slot dst column within window; -1 for dead slots
            colv_np = np.full((128, NW * WCH), -1.0, f16)
            colv_np[slot % 128, slot // 128] = col_of[ed]

            # vrow: local node -> virtual row
            vrow = np.zeros(NTN, np.int64)
            vrow[:NPC] = win_of * 128 + col_of

            idx16_in = np.concatenate(
                [_wrap16_win(src_slots), _wrap16_win(q_slots), _wrap16(vrow)],
                axis=1)
            idxcolv.append(np.ascontiguousarray(np.concatenate(
                [idx16_in.reshape(-1).view(f16), colv_np.reshape(-1)])))
        _graph_cache[gkey] = (NW, idxcolv)
    NW, idxcolv = _graph_cache[gkey]

    key = (NW, use_bias)
    if key not in _cache:
        _cache[key] = _build(NW, use_bias)
    nc = _cache[key]

    # ---- shared input tensors ----
    f16 = np.float16
    f8 = ml_dtypes.float8_e3m4
    wz_full = np.concatenate(
        [Wq_eff / H8SCALE, Wkv_eff / H8SCALE, Wa_eff], axis=1
    ).reshape(2, 128, 4 * D).astype(f16)
    bz_in = np.concatenate([bq_eff, bkv_eff]).astype(f16)

    hlut = _hlut()
    hs16 = (H8SCALE * h).astype(f16)      # [N, D]; LUT clips + casts to e3m4
    in_maps = []
    for c in range(NCORES):
        n0 = c * NPC
        hT16 = np.ascontiguousarray(hs16[n0:n0 + NPC].T)   # [D, NPC]
        hT_in = hlut[hT16.view(np.uint16)]                 # e3m4 bytes
        sections = [
            hT_in.reshape(-1).view(f16),
            wz_full[:, :, c * 128:(c + 1) * 128].reshape(-1),
            idxcolv[c],
        ]
        if use_bias:
            sections.append(bz_in)
        in_maps.append({"blob": np.ascontiguousarray(np.concatenate(sections))})

    import time as _time
    _t0 = _time.perf_counter()
    res = run_bass_kernel_spmd(nc, in_maps, list(range(NCORES)), trace=False)
    LAST_RESULTS = res
    LAST_EXEC_NS = int((_time.perf_counter() - _t0) * 1e9)

    # host-side residual: out = trans + (1-alpha) h + alpha ba
    bres = (alpha * np.asarray(ba, np.float32))[None, :]
    lut = _olut()
    out = np.empty((N, D), np.float32)
    for c in range(NCORES):
        oc = np.asarray(res.results[c]["out"]).reshape(D, NPC)
        out[c * NPC:(c + 1) * NPC] = (
            lut[oc.view(np.uint8)].T
            + ra * h[c * NPC:(c + 1) * NPC] + bres
        )
    return out


# revision 23
# speedup vs baseline: 1.0464x; 1.0464x over previous
"""HGT layer kernel for 8 Trainium2 NeuronCores.

Strategy (dst-sharded graph parallel, transfer-minimized):
  - Host folds relation transforms / priors / skip gate into effective
    weights. h ships as fp8(e3m4, x2-scaled, compensated in the weights),
    transposed; it is widened to fp16 on device for the projections. The
    skip residual (1-alpha)h + alpha*ba is added on the host, where h is
    already resident in fp32.
  - The merged projection weights ship sharded: each core uploads 1/8th and
    the full matrix is rebuilt with a small AllGather before projections.
  - Each core owns N/8=2500 destination nodes and their incoming edges;
    nodes are first-fit-decreasing packed into NW windows of <=128 dst
    nodes / 2048 edge slots.
  - Device: project q/kv for own nodes (fp16), AllGather kv table, then per
    window: dma_gather kv[src] and q[dst] rows, DVE dot-product scores, ACT
    exp, PE onehot-matmul aggregation of [messages | exp] into PSUM,
    normalize, flush. The onehot is built on device (iota + is_equal
    against per-slot column ids), not uploaded.
  - Final: transpose-gather normalized agg; the output projection is
    computed transposed (dout on partitions) and ships back as fp8
    (x16-scaled) [2,128,2500]; host adds the residual and unshards.
  - ALL per-core inputs ship as ONE f16 blob parameter (fp8/int16 sections
    are bitcast-sliced on device); per-array tunnel latency dominates, so
    fewer/larger transfers win. Total traffic ~18MB/call vs 182MB naive.
"""

import math
import os
import numpy as np
import ml_dtypes

import jax

jax.config.update(
    "jax_compilation_cache_dir",
    os.path.join(os.environ.get("TMPDIR", "/tmp"), "bass_hgt_jax_cache"),
)
jax.config.update("jax_persistent_cache_min_compile_time_secs", 0.0)

import concourse.bacc as bacc
import concourse.tile as tile
from concourse import mybir
from concourse.bass_utils import run_bass_kernel_spmd

N = 20000
E = 320000
D = 256
H = 8
DK = 32
NCORES = 8
NPC = N // NCORES          # 2500 nodes per core
NTN = 2560                 # padded nodes per core (20 tiles of 128)
NTILES = NTN // 128        # 20
WSLOTS = 2048              # edge slots per window
WCH = WSLOTS // 128        # 16 chunks per window
WSPAN = 128                # max dst nodes per window
FCH = 512                  # node columns per final-phase chunk
H8SCALE = 2.0              # fp8 pre-scale for h (compensated in wq/wkv)
H8MAX = 15.5               # float8_e3m4 max finite; clip before cast
OSCALE = 16.0              # fp8 scale for the trans output (undone on host)

F16 = mybir.dt.float16
F32 = mybir.dt.float32
F8 = mybir.dt.float8e3
I16 = mybir.dt.int16

# blob section offsets in f16 units
O_HT = 0                                   # [2][128, NPC] fp8
SZ_HT_J = 128 * NPC // 2                   # one j-plane, f16 units
O_WZS = O_HT + 2 * SZ_HT_J                 # [2][128, 128] f16
SZ_WZS_J = 128 * 128

_cache = {}
_graph_cache = {}
LAST_RESULTS = None
LAST_EXEC_NS = None

# fp8(e3m4) byte -> f32/OSCALE lookup for fast output decode
_OLUT = None
# f16 bits -> clipped e3m4 byte lookup for fast h encode
_HLUT = None


def _olut():
    global _OLUT
    if _OLUT is None:
        _OLUT = (
            np.arange(256, dtype=np.uint8).view(ml_dtypes.float8_e3m4)
            .astype(np.float32) * (1.0 / OSCALE)
        )
    return _OLUT


def _hlut():
    global _HLUT
    if _HLUT is None:
        v = np.arange(65536, dtype=np.uint16).view(np.float16).astype(np.float32)
        v = np.clip(np.nan_to_num(v, nan=0.0, posinf=H8MAX, neginf=-H8MAX),
                    -H8MAX, H8MAX)
        _HLUT = v.astype(ml_dtypes.float8_e3m4).view(np.uint8)
    return _HLUT


def _build(NW, use_bias):
    IDXL = 2 * NW * 128 + NTN // 16
    QOFF = NW * 128
    VOFF = 2 * NW * 128
    O_IDX = O_WZS + 2 * SZ_WZS_J           # [16, IDXL] i16
    SZ_IDX = 16 * IDXL
    O_COLV = O_IDX + SZ_IDX                # [128, NW*WCH] int8 (f16 units)
    SZ_COLV = 128 * NW * WCH // 2
    O_BZ = O_COLV + SZ_COLV                # [1, 3D] f16 (optional)
    BLOB = O_BZ + (3 * D if use_bias else 0)

    nc = bacc.Bacc()
    blob = nc.declare_dram_parameter("blob", [BLOB], F16, isOutput=False)
    outp = nc.declare_dram_parameter("out", [2, 128, NPC], F8, isOutput=True)

    with tile.TileContext(nc) as tc:
        with (
            tc.tile_pool(name="const", bufs=1) as constp,
            tc.tile_pool(name="dram", bufs=1, space="DRAM") as dram,
            tc.tile_pool(name="proj", bufs=3) as projp,
            tc.tile_pool(name="psum", bufs=2, space="PSUM") as psump,
            tc.tile_pool(name="edge", bufs=2) as edgep,
            tc.tile_pool(name="fin", bufs=2) as finp,
        ):
            q_tab = dram.tile([NTN, D], F16)
            kv_slice = dram.tile([NTN, 2 * D], F16)
            kv_full = nc.dram_tensor(
                "kv_full", [NCORES * NTN, 2 * D], F16, addr_space="Shared")
            vn = dram.tile([NW * 128, D], F16)
            wzs_dram = dram.tile([2, 128, 128], F16)
            wz_full = nc.dram_tensor(
                "wz_full", [2 * NCORES, 128, 128], F16, addr_space="Shared")

            # ---- weight shard AllGather ----
            wzs_sb = constp.tile([128, 2, 128], F16)
            for j in (0, 1):
                nc.sync.dma_start(
                    wzs_sb[:, j, :],
                    blob[O_WZS + j * SZ_WZS_J:O_WZS + (j + 1) * SZ_WZS_J]
                    .rearrange("(p l) -> p l", p=128))
                nc.sync.dma_start(wzs_dram[j], wzs_sb[:, j, :])
            nc.gpsimd.collective_compute(
                "AllGather",
                mybir.AluOpType.bypass,
                replica_groups=[list(range(NCORES))],
                ins=[wzs_dram.opt()],
                outs=[wz_full[:]],
            )
            wz_sb = constp.tile([128, 2, 4 * D], F16)
            for c8 in range(NCORES):
                for j in (0, 1):
                    nc.sync.dma_start(
                        wz_sb[:, j, c8 * 128:(c8 + 1) * 128], wz_full[c8 * 2 + j])

            # ---- constants ----
            hT8_sb = constp.tile([128, 2, NPC], F8)
            for j in (0, 1):
                nc.sync.dma_start(
                    hT8_sb[:, j, :],
                    blob[O_HT + j * SZ_HT_J:O_HT + (j + 1) * SZ_HT_J]
                    .bitcast(F8).rearrange("(p l) -> p l", p=128))
            hT_sb = constp.tile([128, 2, NTN], F16)
            nc.vector.memset(hT_sb[:], 0.0)
            for j in (0, 1):
                nc.vector.tensor_copy(hT_sb[:, j, 0:NPC], hT8_sb[:, j, :])
            idx_sb = constp.tile([128, IDXL], I16)
            idx_src = blob[O_IDX:O_IDX + SZ_IDX].bitcast(I16).rearrange(
                "(p l) -> p l", p=16)
            for a in range(8):
                nc.sync.dma_start(idx_sb[16 * a:16 * (a + 1), :], idx_src)
            colv8_sb = constp.tile([128, NW * WCH], mybir.dt.int8)
            nc.sync.dma_start(
                colv8_sb[:],
                blob[O_COLV:O_COLV + SZ_COLV].bitcast(mybir.dt.int8)
                .rearrange("(p l) -> p l", p=128))
            colv_sb = constp.tile([128, NW * WCH], F16)
            nc.vector.tensor_copy(colv_sb[:], colv8_sb[:])
            iota_big = constp.tile([128, WCH, 128], F16)
            nc.gpsimd.iota(
                iota_big[:], pattern=[[0, WCH], [1, 128]], base=0,
                channel_multiplier=0, allow_small_or_imprecise_dtypes=True)
            if use_bias:
                ones_sb = constp.tile([1, 128], F16)
                nc.vector.memset(ones_sb[:], 1.0)
                bz_sb = constp.tile([1, 3 * D], F16)
                nc.sync.dma_start(
                    bz_sb[:], blob[O_BZ:O_BZ + 3 * D].rearrange("(p l) -> p l", p=1))

            # ---- projection phase ----
            for nt in range(NTILES):
                sl = slice(nt * 128, (nt + 1) * 128)
                pkv = psump.tile([128, 2 * D], F32, tag="pkv")
                for j in (0, 1):
                    nc.tensor.matmul(
                        pkv[:], hT_sb[:, j, sl], wz_sb[:, j, D:3 * D],
                        start=(j == 0), stop=(j == 1 and not use_bias),
                    )
                if use_bias:
                    nc.tensor.matmul(
                        pkv[:], ones_sb[:], bz_sb[:, D:3 * D], start=False, stop=True)
                kv_sb = projp.tile([128, 2 * D], F16, tag="kv")
                nc.vector.tensor_copy(kv_sb[:], pkv[:])
                nc.sync.dma_start(kv_slice[sl, :], kv_sb[:])

                pq = psump.tile([128, D], F32, tag="pq")
                for j in (0, 1):
                    nc.tensor.matmul(
                        pq[:], hT_sb[:, j, sl], wz_sb[:, j, 0:D],
                        start=(j == 0), stop=(j == 1 and not use_bias),
                    )
                if use_bias:
                    nc.tensor.matmul(
                        pq[:], ones_sb[:], bz_sb[:, 0:D], start=False, stop=True)
                q_sb = projp.tile([128, D], F16, tag="q")
                nc.vector.tensor_copy(q_sb[:], pq[:])
                nc.sync.dma_start(q_tab[sl, :], q_sb[:])

            nc.gpsimd.collective_compute(
                "AllGather",
                mybir.AluOpType.bypass,
                replica_groups=[list(range(NCORES))],
                ins=[kv_slice.opt()],
                outs=[kv_full[:]],
            )

            # ---- edge phase ----
            for w in range(NW):
                csl = slice(w * 128, (w + 1) * 128)
                kvg = edgep.tile([128, WCH, 2 * D], F16, tag="kvg")
                nc.gpsimd.dma_gather(
                    kvg[:], kv_full[:], idx_sb[:, csl],
                    num_idxs=WSLOTS, num_idxs_reg=WSLOTS, elem_size=2 * D,
                    single_packet=False,
                )
                qg = edgep.tile([128, WCH, D], F16, tag="qg")
                nc.gpsimd.dma_gather(
                    qg[:], q_tab[:], idx_sb[:, QOFF + w * 128:QOFF + (w + 1) * 128],
                    num_idxs=WSLOTS, num_idxs_reg=WSLOTS, elem_size=D,
                    single_packet=False,
                )
                oa_sb = edgep.tile([128, WCH, 128], F16, tag="oa")
                nc.vector.tensor_tensor(
                    oa_sb[:],
                    colv_sb[:, w * WCH:(w + 1) * WCH].broadcast_to([128, WCH, 128]),
                    iota_big[:],
                    op=mybir.AluOpType.is_equal,
                )

                prod = edgep.tile([128, WCH, D], F16, tag="prod")
                nc.vector.tensor_mul(prod[:], qg[:], kvg[:, :, 0:D])
                scores = edgep.tile([128, WCH, H], F32, tag="sc")
                nc.vector.tensor_reduce(
                    scores[:],
                    prod[:].rearrange("p c (h k) -> p c h k", h=H),
                    axis=mybir.AxisListType.X,
                    op=mybir.AluOpType.add,
                )
                msgz = edgep.tile([128, WCH, D + H], F16, tag="msgz")
                nc.scalar.activation(
                    msgz[:, :, D:D + H], scores[:], mybir.ActivationFunctionType.Exp
                )
                nc.vector.tensor_mul(
                    msgz[:, :, 0:D].rearrange("p c (h k) -> p c h k", h=H),
                    kvg[:, :, D:2 * D].rearrange("p c (h k) -> p c h k", h=H),
                    msgz[:, :, D:D + H].broadcast_to([128, WCH, H, DK]),
                )
                pw = psump.tile([128, D + H], F32, tag="pw")
                for i in range(WCH):
                    nc.tensor.matmul(
                        pw[:], oa_sb[:, i, :], msgz[:, i, :],
                        start=(i == 0), stop=(i == WCH - 1),
                    )
                zr = finp.tile([128, H], F32, tag="zr")
                nc.vector.tensor_scalar_add(zr[:], pw[:, D:D + H], 1e-30)
                zrec = finp.tile([128, H], F32, tag="zrec")
                nc.vector.reciprocal(zrec[:], zr[:])
                vb = finp.tile([128, D], F16, tag="vb")
                nc.vector.tensor_mul(
                    vb[:].rearrange("p (h k) -> p h k", h=H),
                    pw[:, 0:D].rearrange("p (h k) -> p h k", h=H),
                    zrec[:].broadcast_to([128, H, DK]),
                )
                nc.sync.dma_start(vn[csl, :], vb[:])

            # ---- final phase (transposed: dout on partitions) ----
            tg = constp.tile([128, 2, NTN], F16)
            nc.gpsimd.dma_gather(
                tg[:], vn[:], idx_sb[:, VOFF:VOFF + NTN // 16],
                num_idxs=NTN, num_idxs_reg=NTN, elem_size=D, transpose=True,
                single_packet=False,
            )
            for half in (0, 1):
                wsl = slice(3 * D + half * 128, 3 * D + (half + 1) * 128)
                for c0 in range(0, NPC, FCH):
                    c1 = min(c0 + FCH, NPC)
                    cw = c1 - c0
                    po = psump.tile([128, FCH], F32, tag="po")
                    for j in (0, 1):
                        nc.tensor.matmul(
                            po[:, 0:cw], wz_sb[:, j, wsl], tg[:, j, c0:c1],
                            start=(j == 0), stop=(j == 1),
                        )
                    # scale into fp8 range, clamp both sides, cast to e3m4
                    oth = finp.tile([128, FCH], F16, tag="oth")
                    nc.vector.tensor_scalar(
                        oth[:, 0:cw], po[:, 0:cw], OSCALE, H8MAX,
                        op0=mybir.AluOpType.mult, op1=mybir.AluOpType.min,
                    )
                    ot8 = finp.tile([128, FCH], F8, tag="ot8")
                    nc.vector.tensor_scalar_max(ot8[:, 0:cw], oth[:, 0:cw], -H8MAX)
                    nc.sync.dma_start(outp[half, :, c0:c1], ot8[:, 0:cw])

    nc.compile()
    return nc


def _wrap16(v):
    """[L] int array -> [16, L//16] wrapped int16 (16-partition wrap)."""
    L = v.shape[0]
    return np.ascontiguousarray(v.reshape(L // 16, 16).T.astype(np.int16))


def _wrap16_win(v):
    """[NW, WSLOTS] -> [16, NW*128]: per-window wrapped layout."""
    NW = v.shape[0]
    return np.ascontiguousarray(
        v.reshape(NW, WSLOTS // 16, 16)
        .transpose(2, 0, 1)
        .reshape(16, NW * (WSLOTS // 16))
        .astype(np.int16)
    )


def _pack_windows(degs):
    """Next-fit-decreasing pack nodes into windows of <=WSPAN nodes /
    <=WSLOTS slots. Returns (win_of, col_of, n_windows)."""
    npc = degs.shape[0]
    assert degs.max() <= WSLOTS, "node degree exceeds window slot capacity"
    order = np.argsort(-degs, kind="stable")
    cum = np.cumsum(degs[order])
    win_of_s = np.empty(npc, np.int64)
    col_of_s = np.empty(npc, np.int64)
    start = 0
    base = 0
    w = 0
    while start < npc:
        hi = min(start + WSPAN, npc)
        m = int(np.searchsorted(cum[start:hi], base + WSLOTS, side="right"))
        assert m > 0
        win_of_s[start:start + m] = w
        col_of_s[start:start + m] = np.arange(m)
        base = int(cum[start + m - 1])
        start += m
        w += 1
    win_of = np.empty(npc, np.int64)
    col_of = np.empty(npc, np.int64)
    win_of[order] = win_of_s
    col_of[order] = col_of_s
    return win_of, col_of, w


def kernel(h, src, dst, Wk, bk, Wq, bq, Wv, bv, Wa, ba, rel_att, rel_msg, rel_pri, skip):
    global LAST_RESULTS, LAST_EXEC_NS
    h = np.asarray(h, np.float32)
    src = np.asarray(src, np.int32)
    dst = np.asarray(dst, np.int32)

    # ---- fold weights on host ----
    scale = (np.asarray(rel_pri, np.float32) / math.sqrt(DK)).astype(np.float32)
    WqT = np.asarray(Wq, np.float32).T.reshape(D, H, DK)
    Wq_eff = (WqT * scale[None, :, None]).reshape(D, D)
    bq_eff = (np.asarray(bq, np.float32).reshape(H, DK) * scale[:, None]).reshape(D)
    WkT = np.asarray(Wk, np.float32).T.reshape(D, H, DK)
    Wk_eff = np.einsum("dhk,hke->dhe", WkT, np.asarray(rel_att, np.float32)).reshape(D, D)
    bk_eff = np.einsum("hk,hke->he", np.asarray(bk, np.float32).reshape(H, DK),
                       np.asarray(rel_att, np.float32)).reshape(D)
    WvT = np.asarray(Wv, np.float32).T.reshape(D, H, DK)
    Wv_eff = np.einsum("dhk,hke->dhe", WvT, np.asarray(rel_msg, np.float32)).reshape(D, D)
    bv_eff = np.einsum("hk,hke->he", np.asarray(bv, np.float32).reshape(H, DK),
                       np.asarray(rel_msg, np.float32)).reshape(D)
    Wkv_eff = np.concatenate([Wk_eff, Wv_eff], axis=1)          # [256, 512]
    bkv_eff = np.concatenate([bk_eff, bv_eff])                  # [512]
    alpha = float(1.0 / (1.0 + math.exp(-float(np.asarray(skip)))))
    ra = 1.0 - alpha
    Wa_eff = (alpha * np.asarray(Wa, np.float32).T)             # [256, 256]
    use_bias = bool(np.any(bq_eff) or np.any(bkv_eff))

    # ---- edge preprocessing (memoized on graph content) ----
    import hashlib
    gk = hashlib.blake2b(src.tobytes(), digest_size=16)
    gk.update(dst.tobytes())
    gkey = gk.digest()
    if gkey not in _graph_cache:
        order = np.argsort(dst, kind="stable")
        dsts = dst[order]
        srcs = src[order]
        core_of = dsts // NPC
        core_starts = np.searchsorted(core_of, np.arange(NCORES + 1))
        deg = np.bincount(dst, minlength=N)

        packs = []
        NW = 0
        for c in range(NCORES):
            win_of, col_of, nw = _pack_windows(deg[c * NPC:(c + 1) * NPC])
            packs.append((win_of, col_of))
            NW = max(NW, nw)

        f16 = np.float16
        idxcolv = []
        for c in range(NCORES):
            n0 = c * NPC
            e0, e1 = core_starts[c], core_starts[c + 1]
            ed = dsts[e0:e1] - n0         # local dst
            es = srcs[e0:e1]              # global src
            win_of, col_of = packs[c]
            wid = win_of[ed]
            # slot assignment: within window, sort by src kv row for locality
            es_row = (es // NPC) * NTN + (es % NPC)
            sort2 = np.lexsort((es_row, wid))
            ed = ed[sort2]
            es_row = es_row[sort2]
            wid = wid[sort2]
            # rank within window
            wcounts = np.bincount(wid, minlength=NW)
            woff = np.zeros(NW + 1, np.int64)
            np.cumsum(wcounts, out=woff[1:])
            rank = np.arange(e1 - e0) - woff[wid]
            slot = wid * WSLOTS + rank    # global slot in [0, NW*WSLOTS)

            src_slots = np.zeros((NW, WSLOTS), np.int64)
            q_slots = np.zeros((NW, WSLOTS), np.int64)
            src_slots.reshape(-1)[slot] = es_row
            q_slots.reshape(-1)[slot] = ed
            # per-slot dst column within window; -1 for dead slots
            colv_np = np.full((128, NW * WCH), -1, np.int8)
            colv_np[slot % 128, slot // 128] = col_of[ed]

            # vrow: local node -> virtual row
            vrow = np.zeros(NTN, np.int64)
            vrow[:NPC] = win_of * 128 + col_of

            idx16_in = np.concatenate(
                [_wrap16_win(src_slots), _wrap16_win(q_slots), _wrap16(vrow)],
                axis=1)
            idxcolv.append(np.ascontiguousarray(np.concatenate(
                [idx16_in.reshape(-1).view(f16), colv_np.reshape(-1).view(f16)])))
        _graph_cache[gkey] = (NW, idxcolv)
    NW, idxcolv = _graph_cache[gkey]

    key = (NW, use_bias)
    if key not in _cache:
        _cache[key] = _build(NW, use_bias)
    nc = _cache[key]

    # ---- shared input tensors ----
    f16 = np.float16
    f8 = ml_dtypes.float8_e3m4
    wz_full = np.concatenate(
        [Wq_eff / H8SCALE, Wkv_eff / H8SCALE, Wa_eff], axis=1
    ).reshape(2, 128, 4 * D).astype(f16)
    bz_in = np.concatenate([bq_eff, bkv_eff]).astype(f16)

    hlut = _hlut()
    hs16 = (H8SCALE * h).astype(f16)      # [N, D]; LUT clips + casts to e3m4
    in_maps = []
    for c in range(NCORES):
        n0 = c * NPC
        hT16 = np.ascontiguousarray(hs16[n0:n0 + NPC].T)   # [D, NPC]
        hT_in = hlut[hT16.view(np.uint16)]                 # e3m4 bytes
        sections = [
            hT_in.reshape(-1).view(f16),
            wz_full[:, :, c * 128:(c + 1) * 128].reshape(-1),
            idxcolv[c],
        ]
        if use_bias:
            sections.append(bz_in)
        in_maps.append({"blob": np.ascontiguousarray(np.concatenate(sections))})

    import time as _time
    _t0 = _time.perf_counter()
    res = run_bass_kernel_spmd(nc, in_maps, list(range(NCORES)), trace=False)
    LAST_RESULTS = res
    LAST_EXEC_NS = int((_time.perf_counter() - _t0) * 1e9)

    # host-side residual: out = trans + (1-alpha) h + alpha ba
    bres = (alpha * np.asarray(ba, np.float32))[None, :]
    lut = _olut()
    out = np.empty((N, D), np.float32)
    for c in range(NCORES):
        oc = np.asarray(res.results[c]["out"]).reshape(D, NPC)
        out[c * NPC:(c + 1) * NPC] = (
            lut[oc.view(np.uint8)].T
            + ra * h[c * NPC:(c + 1) * NPC] + bres
        )
    return out
